# revision 10
# baseline (speedup 1.0000x reference)
"""DropSphereNd Trainium2 kernel.

Full computation (per sample n, channels c):
    activ = embeds @ table                      # [n, c]
    t     = 17th-smallest(activ, axis=1)        # [n, 1]
    out   = x * (activ >= t) * c/(c-16)

Sharding: data-parallel over batch n across 8 cores (x/embeds sharded,
table replicated).  Per core: x shard [8, 256, 56, 56] viewed as
[2048, 3136]; the mask is computed on-device (tiny matmul + iterative
min-extraction) and applied as a per-partition scalar multiply while
streaming x through SBUF.

Raw bass (no Tile): the pinned walrus codegen allows only ONE sync-wait
per compute instruction, so all cross-engine deps use standalone
wait_ge sequencer commands.

Engine plan:
  SP  (nc.sync)   - input DMAs (table, embT, ident, x tiles)
  ACT (nc.scalar) - output DMAs
  PE  (nc.tensor) - projection matmul + 2 mask-transpose matmuls
  DVE (nc.vector) - threshold search, mask build, streaming multiplies
"""

import sys

if "/opt/trn_rl_repo" not in sys.path:
    sys.path.insert(0, "/opt/trn_rl_repo")

from contextlib import ExitStack

import numpy as np

import concourse.bass as bass
from concourse import mybir
from concourse.bass_utils import run_bass_kernel_spmd

N, C, H, W = 64, 256, 56, 56
HW = H * W  # 3136
E = 16
NCORES = 8
NLOC = N // NCORES  # 8 samples per core
INDEX = 16  # ceil(C ** 0.5)
SCALE = float(C) / (C - INDEX)
F32 = mybir.dt.float32
BIG = 1.0e30
BUFS = 4  # x-tile ring slots (25 KB/partition each)

_NC_CACHE = {}


def _build_nc() -> bass.Bass:
    # detect_race_conditions only affects the interpreter: its raw-bass model
    # has no same-engine program-order edges, so every chained DVE op would be
    # flagged.  Cross-engine ordering is handled by the explicit sems below.
    nc = bass.Bass(detect_race_conditions=False)
    x = nc.dram_tensor("x", [NLOC * C, HW], F32, kind="ExternalInput")
    emb = nc.dram_tensor("embeds", [NLOC, E], F32, kind="ExternalInput")
    tab = nc.dram_tensor("table", [E, C], F32, kind="ExternalInput")
    out = nc.dram_tensor("out", [NLOC * C, HW], F32, kind="ExternalOutput")
    ident_d = nc.inline_tensor(np.eye(NLOC, dtype=np.float32), name="ident8")

    # row r = t*256 + 2*p + parity  ->  sample t, channel c = 2*p + parity
    x_t = x[:, :].rearrange("(t p two) f -> t p two f", p=128, two=2)
    o_t = out[:, :].rearrange("(t p two) f -> t p two f", p=128, two=2)

    with ExitStack() as ctx:
        sb = lambda name, shape: ctx.enter_context(nc.sbuf_tensor(name, shape, F32))
        ps = lambda name, shape: ctx.enter_context(nc.psum_tensor(name, shape, F32))

        tab_s = sb("tab_s", [E, C])
        embT = sb("embT", [E, NLOC])
        ident = sb("ident", [NLOC, NLOC])
        a = sb("a", [NLOC, C])
        w = sb("w", [NLOC, C])
        tmp = sb("tmp", [NLOC, C])
        mn = sb("mn", [NLOC, 1])
        thr = sb("thr", [NLOC, 1])
        m_even = sb("m_even", [NLOC, C // 2])
        m_odd = sb("m_odd", [NLOC, C // 2])
        mA = sb("mA", [C // 2, NLOC])
        mB = sb("mB", [C // 2, NLOC])
        xbuf = [sb(f"xbuf{i}", [128, 2, HW]) for i in range(BUFS)]

        activ_p = ps("activ_p", [NLOC, C])
        mA_p = ps("mA_p", [C // 2, NLOC])
        mB_p = ps("mB_p", [C // 2, NLOC])

        ld = ctx.enter_context(nc.semaphore("ld"))
        dv = ctx.enter_context(nc.semaphore("dv"))
        pe = ctx.enter_context(nc.semaphore("pe"))
        # per-ring-slot DMA sems: same-sem increments are serialized by the
        # slot lifecycle, so wait values are unambiguous (race-detector clean)
        xs = [ctx.enter_context(nc.semaphore(f"xs{i}")) for i in range(BUFS)]
        ss = [ctx.enter_context(nc.semaphore(f"ss{i}")) for i in range(BUFS)]

        block = ctx.enter_context(nc.Block())

        @block.sync
        def _(sync):
            sync.dma_start(out=tab_s[:, :], in_=tab[:, :]).then_inc(ld, 16)
            with nc.allow_non_contiguous_dma(reason="8x16 transposed load, 512B"):
                sync.dma_start(
                    out=embT[:, :], in_=emb[:, :].rearrange("n e -> e n")
                ).then_inc(ld, 16)
            sync.dma_start(out=ident[:, :], in_=ident_d[:, :]).then_inc(ld, 16)
            for t in range(NLOC):
                if t >= BUFS:
                    # slot free once the store of tile t-BUFS has drained
                    sync.wait_ge(ss[t % BUFS], 16 * (t // BUFS))
                sync.dma_start(out=xbuf[t % BUFS][:, :, :], in_=x_t[t]).then_inc(
                    xs[t % BUFS], 16
                )

        @block.tensor
        def _(tensor):
            tensor.wait_ge(ld, 48)  # tab_s + embT (+ident) resident
            tensor.matmul(
                activ_p[:, :], embT[:, :], tab_s[:, :], start=True, stop=True
            ).then_inc(pe, 1)
            tensor.wait_ge(dv, INDEX + 3)  # m_even + m_odd built
            tensor.matmul(
                mA_p[:, :], m_even[:, :], ident[:, :], start=True, stop=True
            ).then_inc(pe, 1)
            tensor.matmul(
                mB_p[:, :], m_odd[:, :], ident[:, :], start=True, stop=True
            ).then_inc(pe, 1)

        # TensorScalarPtr fetches its scalar operand at sequencer dispatch,
        # ahead of the DVE pipe -- every freshly-written scalar (mn, thr,
        # mA/mB) must be sem-fenced before the consuming instruction, even
        # same-engine.  dvv tracks the dv sem value at trace time.
        dvv = 0

        @block.vector
        def _(vector):
            nonlocal_dvv = dvv  # noqa: F841 (trace-time bookkeeping below)
            v = 0
            vector.wait_ge(pe, 1)
            vector.tensor_copy(a[:, :], activ_p[:, :])
            vector.tensor_copy(w[:, :], activ_p[:, :])
            # knock out the 16 smallest; the 17th-smallest remains as min
            for _ in range(INDEX):
                vector.tensor_reduce(
                    mn[:, :], w[:, :], axis=mybir.AxisListType.X, op=mybir.AluOpType.min
                ).then_inc(dv, 1)
                v += 1
                vector.wait_ge(dv, v)  # mn committed before its ptr-fetch
                vector.tensor_scalar(
                    out=tmp[:, :],
                    in0=w[:, :],
                    scalar1=mn[:, :],
                    scalar2=BIG,
                    op0=mybir.AluOpType.is_equal,
                    op1=mybir.AluOpType.mult,
                )
                vector.tensor_add(w[:, :], w[:, :], tmp[:, :])
            vector.tensor_reduce(
                thr[:, :], w[:, :], axis=mybir.AxisListType.X, op=mybir.AluOpType.min
            ).then_inc(dv, 1)
            v += 1
            vector.wait_ge(dv, v)  # thr committed
            # mask[n, c] = (activ >= thr) * SCALE, split by channel parity
            a_pair = a[:, :].rearrange("n (j two) -> n j two", two=2)
            vector.tensor_scalar(
                out=m_even[:, :],
                in0=a_pair[:, :, 0],
                scalar1=thr[:, :],
                scalar2=SCALE,
                op0=mybir.AluOpType.is_ge,
                op1=mybir.AluOpType.mult,
            ).then_inc(dv, 1)
            v += 1
            vector.tensor_scalar(
                out=m_odd[:, :],
                in0=a_pair[:, :, 1],
                scalar1=thr[:, :],
                scalar2=SCALE,
                op0=mybir.AluOpType.is_ge,
                op1=mybir.AluOpType.mult,
            ).then_inc(dv, 1)
            v += 1
            vector.wait_ge(pe, 3)
            vector.tensor_copy(mA[:, :], mA_p[:, :])
            vector.tensor_copy(mB[:, :], mB_p[:, :]).then_inc(dv, 1)
            v += 1
            vector.wait_ge(dv, v)  # mA/mB committed before mul ptr-fetches
            for t in range(NLOC):
                vector.wait_ge(xs[t % BUFS], 16 * (t // BUFS + 1))
                xb = xbuf[t % BUFS]
                vector.tensor_scalar_mul(
                    xb[:, 0, :], xb[:, 0, :], mA[:, t : t + 1]
                )
                vector.tensor_scalar_mul(
                    xb[:, 1, :], xb[:, 1, :], mB[:, t : t + 1]
                ).then_inc(dv, 1)

        # dv value right before the streaming muls:
        # 16 (mn) + 1 (thr) + 2 (masks) + 1 (mA/mB copies) = 20
        DV_MASKS = INDEX + 3  # m_even+m_odd done (PE gate)
        DV_BASE = INDEX + 4   # after mA/mB copy inc

        @block.scalar
        def _(scalar):
            for t in range(NLOC):
                scalar.wait_ge(dv, DV_BASE + (t + 1))  # both muls of tile t done
                scalar.dma_start(out=o_t[t], in_=xbuf[t % BUFS][:, :, :]).then_inc(
                    ss[t % BUFS], 16
                )

    return nc


def _get_nc() -> bass.Bass:
    if "nc" not in _NC_CACHE:
        _NC_CACHE["nc"] = _build_nc()
    return _NC_CACHE["nc"]


def _in_maps(x, embeds, table):
    x = np.ascontiguousarray(np.asarray(x, dtype=np.float32))
    embeds = np.ascontiguousarray(np.asarray(embeds, dtype=np.float32))
    table = np.ascontiguousarray(np.asarray(table, dtype=np.float32))
    maps = []
    for i in range(NCORES):
        maps.append(
            {
                "x": x[i * NLOC : (i + 1) * NLOC].reshape(NLOC * C, HW),
                "embeds": embeds[i * NLOC : (i + 1) * NLOC],
                "table": table,
            }
        )
    return maps


def kernel(x, embeds, table):
    nc = _get_nc()
    res = run_bass_kernel_spmd(nc, _in_maps(x, embeds, table), list(range(NCORES)))
    shards = [
        np.asarray(res.results[i]["out"]).reshape(NLOC, C, H, W)
        for i in range(NCORES)
    ]
    return np.concatenate(shards, axis=0)


def kernel_profiled(x, embeds, table, **trace_kwargs):
    """Same as kernel() but with NTFF tracing; returns (output, BassKernelResults)."""
    nc = _get_nc()
    res = run_bass_kernel_spmd(
        nc, _in_maps(x, embeds, table), list(range(NCORES)), trace=True, **trace_kwargs
    )
    shards = [
        np.asarray(res.results[i]["out"]).reshape(NLOC, C, H, W)
        for i in range(NCORES)
    ]
    return np.concatenate(shards, axis=0), res


# revision 13
# speedup vs baseline: 1.2580x; 1.2580x over previous
"""DropSphereNd Trainium2 kernel.

Full computation (per sample n, channels c):
    activ = embeds @ table                      # [n, c]
    t     = 17th-smallest(activ, axis=1)        # [n, 1]
    out   = x * (activ >= t) * c/(c-16)

Sharding: data-parallel over batch n across 8 cores (x/embeds sharded,
table replicated).  Per core: x shard [8, 256, 56, 56] viewed as
[2048, 3136]; the mask is computed on-device (tiny matmul + iterative
min-extraction) and applied as a per-partition scalar multiply while
streaming x through SBUF.

Raw bass (no Tile): the pinned walrus codegen allows only ONE sync-wait
per compute instruction, so all cross-engine deps use standalone
wait_ge sequencer commands.

Engine plan:
  SP  (nc.sync)   - input DMAs (table, embT, ident, x tiles)
  ACT (nc.scalar) - output DMAs
  PE  (nc.tensor) - projection matmul + 2 mask-transpose matmuls
  DVE (nc.vector) - threshold search, mask build, streaming multiplies
"""

import sys

if "/opt/trn_rl_repo" not in sys.path:
    sys.path.insert(0, "/opt/trn_rl_repo")

from contextlib import ExitStack

import numpy as np

import concourse.bass as bass
from concourse import mybir
from concourse.bass_utils import run_bass_kernel_spmd

N, C, H, W = 64, 256, 56, 56
HW = H * W  # 3136
E = 16
NCORES = 8
NLOC = N // NCORES  # 8 samples per core
INDEX = 16  # ceil(C ** 0.5)
SCALE = float(C) / (C - INDEX)
F32 = mybir.dt.float32
BIG = 1.0e30
BUFS = 6  # x-tile ring slots (25 KB/partition each)

_NC_CACHE = {}


def _build_nc() -> bass.Bass:
    # detect_race_conditions only affects the interpreter: its raw-bass model
    # has no same-engine program-order edges, so every chained DVE op would be
    # flagged.  Cross-engine ordering is handled by the explicit sems below.
    nc = bass.Bass(detect_race_conditions=False)
    x = nc.dram_tensor("x", [NLOC * C, HW], F32, kind="ExternalInput")
    emb = nc.dram_tensor("embeds", [NLOC, E], F32, kind="ExternalInput")
    tab = nc.dram_tensor("table", [E, C], F32, kind="ExternalInput")
    out = nc.dram_tensor("out", [NLOC * C, HW], F32, kind="ExternalOutput")
    ident_d = nc.inline_tensor(np.eye(NLOC, dtype=np.float32), name="ident8")

    # row r = t*256 + 2*p + parity  ->  sample t, channel c = 2*p + parity
    x_t = x[:, :].rearrange("(t p two) f -> t p two f", p=128, two=2)
    o_t = out[:, :].rearrange("(t p two) f -> t p two f", p=128, two=2)

    with ExitStack() as ctx:
        sb = lambda name, shape: ctx.enter_context(nc.sbuf_tensor(name, shape, F32))
        ps = lambda name, shape: ctx.enter_context(nc.psum_tensor(name, shape, F32))

        tab_s = sb("tab_s", [E, C])
        embT = sb("embT", [E, NLOC])
        ident = sb("ident", [NLOC, NLOC])
        a = sb("a", [NLOC, C])
        w = sb("w", [NLOC, C])
        tmp = sb("tmp", [NLOC, C])
        mn = sb("mn", [NLOC, 1])
        thr = sb("thr", [NLOC, 1])
        m_even = sb("m_even", [NLOC, C // 2])
        m_odd = sb("m_odd", [NLOC, C // 2])
        mA = sb("mA", [C // 2, NLOC])
        mB = sb("mB", [C // 2, NLOC])
        xbuf = [sb(f"xbuf{i}", [128, 2, HW]) for i in range(BUFS)]

        activ_p = ps("activ_p", [NLOC, C])
        mA_p = ps("mA_p", [C // 2, NLOC])
        mB_p = ps("mB_p", [C // 2, NLOC])

        ld = ctx.enter_context(nc.semaphore("ld"))
        dv = ctx.enter_context(nc.semaphore("dv"))
        pe = ctx.enter_context(nc.semaphore("pe"))
        # per-ring-slot DMA sems: same-sem increments are serialized by the
        # slot lifecycle, so wait values are unambiguous (race-detector clean)
        xs = [ctx.enter_context(nc.semaphore(f"xs{i}")) for i in range(BUFS)]
        ss = [ctx.enter_context(nc.semaphore(f"ss{i}")) for i in range(BUFS)]

        block = ctx.enter_context(nc.Block())

        @block.sync
        def _(sync):
            sync.dma_start(out=tab_s[:, :], in_=tab[:, :]).then_inc(ld, 16)
            with nc.allow_non_contiguous_dma(reason="8x16 transposed load, 512B"):
                sync.dma_start(
                    out=embT[:, :], in_=emb[:, :].rearrange("n e -> e n")
                ).then_inc(ld, 16)
            sync.dma_start(out=ident[:, :], in_=ident_d[:, :]).then_inc(ld, 16)
            for t in range(NLOC):
                if t >= BUFS:
                    # slot free once the store of tile t-BUFS has drained
                    sync.wait_ge(ss[t % BUFS], 16 * (t // BUFS))
                sync.dma_start(out=xbuf[t % BUFS][:, :, :], in_=x_t[t]).then_inc(
                    xs[t % BUFS], 16
                )

        @block.tensor
        def _(tensor):
            tensor.wait_ge(ld, 48)  # tab_s + embT (+ident) resident
            tensor.matmul(
                activ_p[:, :], embT[:, :], tab_s[:, :], start=True, stop=True
            ).then_inc(pe, 1)
            tensor.wait_ge(dv, INDEX + 3)  # m_even + m_odd built
            tensor.matmul(
                mA_p[:, :], m_even[:, :], ident[:, :], start=True, stop=True
            ).then_inc(pe, 1)
            tensor.matmul(
                mB_p[:, :], m_odd[:, :], ident[:, :], start=True, stop=True
            ).then_inc(pe, 1)

        # TensorScalarPtr fetches its scalar operand at sequencer dispatch,
        # ahead of the DVE pipe -- every freshly-written scalar (mn, thr,
        # mA/mB) must be sem-fenced before the consuming instruction, even
        # same-engine.  dvv tracks the dv sem value at trace time.
        dvv = 0

        @block.vector
        def _(vector):
            nonlocal_dvv = dvv  # noqa: F841 (trace-time bookkeeping below)
            v = 0
            vector.wait_ge(pe, 1)
            vector.tensor_copy(a[:, :], activ_p[:, :])
            vector.tensor_copy(w[:, :], activ_p[:, :])
            # knock out the 16 smallest; the 17th-smallest remains as min
            for _ in range(INDEX):
                vector.tensor_reduce(
                    mn[:, :], w[:, :], axis=mybir.AxisListType.X, op=mybir.AluOpType.min
                ).then_inc(dv, 1)
                v += 1
                vector.wait_ge(dv, v)  # mn committed before its ptr-fetch
                vector.tensor_scalar(
                    out=tmp[:, :],
                    in0=w[:, :],
                    scalar1=mn[:, :],
                    scalar2=BIG,
                    op0=mybir.AluOpType.is_equal,
                    op1=mybir.AluOpType.mult,
                )
                vector.tensor_add(w[:, :], w[:, :], tmp[:, :])
            vector.tensor_reduce(
                thr[:, :], w[:, :], axis=mybir.AxisListType.X, op=mybir.AluOpType.min
            ).then_inc(dv, 1)
            v += 1
            vector.wait_ge(dv, v)  # thr committed
            # mask[n, c] = (activ >= thr) * SCALE, split by channel parity
            a_pair = a[:, :].rearrange("n (j two) -> n j two", two=2)
            vector.tensor_scalar(
                out=m_even[:, :],
                in0=a_pair[:, :, 0],
                scalar1=thr[:, :],
                scalar2=SCALE,
                op0=mybir.AluOpType.is_ge,
                op1=mybir.AluOpType.mult,
            ).then_inc(dv, 1)
            v += 1
            vector.tensor_scalar(
                out=m_odd[:, :],
                in0=a_pair[:, :, 1],
                scalar1=thr[:, :],
                scalar2=SCALE,
                op0=mybir.AluOpType.is_ge,
                op1=mybir.AluOpType.mult,
            ).then_inc(dv, 1)
            v += 1
            vector.wait_ge(pe, 3)
            vector.tensor_copy(mA[:, :], mA_p[:, :])
            vector.tensor_copy(mB[:, :], mB_p[:, :]).then_inc(dv, 1)
            v += 1
            vector.wait_ge(dv, v)  # mA/mB committed before mul ptr-fetches
            for t in range(NLOC):
                vector.wait_ge(xs[t % BUFS], 16 * (t // BUFS + 1))
                xb = xbuf[t % BUFS]
                vector.tensor_scalar_mul(
                    xb[:, 0, :], xb[:, 0, :], mA[:, t : t + 1]
                )
                vector.tensor_scalar_mul(
                    xb[:, 1, :], xb[:, 1, :], mB[:, t : t + 1]
                ).then_inc(dv, 1)

        # dv value right before the streaming muls:
        # 16 (mn) + 1 (thr) + 2 (masks) + 1 (mA/mB copies) = 20
        DV_MASKS = INDEX + 3  # m_even+m_odd done (PE gate)
        DV_BASE = INDEX + 4   # after mA/mB copy inc

        @block.scalar
        def _(scalar):
            for t in range(NLOC):
                scalar.wait_ge(dv, DV_BASE + (t + 1))  # both muls of tile t done
                scalar.dma_start(out=o_t[t], in_=xbuf[t % BUFS][:, :, :]).then_inc(
                    ss[t % BUFS], 16
                )

    return nc


def _get_nc() -> bass.Bass:
    if "nc" not in _NC_CACHE:
        _NC_CACHE["nc"] = _build_nc()
    return _NC_CACHE["nc"]


def _in_maps(x, embeds, table):
    x = np.ascontiguousarray(np.asarray(x, dtype=np.float32))
    embeds = np.ascontiguousarray(np.asarray(embeds, dtype=np.float32))
    table = np.ascontiguousarray(np.asarray(table, dtype=np.float32))
    maps = []
    for i in range(NCORES):
        maps.append(
            {
                "x": x[i * NLOC : (i + 1) * NLOC].reshape(NLOC * C, HW),
                "embeds": embeds[i * NLOC : (i + 1) * NLOC],
                "table": table,
            }
        )
    return maps


def kernel(x, embeds, table):
    nc = _get_nc()
    res = run_bass_kernel_spmd(nc, _in_maps(x, embeds, table), list(range(NCORES)))
    shards = [
        np.asarray(res.results[i]["out"]).reshape(NLOC, C, H, W)
        for i in range(NCORES)
    ]
    return np.concatenate(shards, axis=0)


def kernel_profiled(x, embeds, table, **trace_kwargs):
    """Same as kernel() but with NTFF tracing; returns (output, BassKernelResults)."""
    nc = _get_nc()
    res = run_bass_kernel_spmd(
        nc, _in_maps(x, embeds, table), list(range(NCORES)), trace=True, **trace_kwargs
    )
    shards = [
        np.asarray(res.results[i]["out"]).reshape(NLOC, C, H, W)
        for i in range(NCORES)
    ]
    return np.concatenate(shards, axis=0), res
